# revision 21
# baseline (speedup 1.0000x reference)
"""Trainium2 Bass kernel for the KAN-to-MLP module.

Math: out = GELU( silu(x) @ base_w.T + einsum('nhk,ohk->no', bsplines(x), spline_w * scaler) )

Reformulation: both branches fuse into one PSUM accumulation with
contraction K = H (silu branch, bf16) + 8*H (8 B-spline basis planes,
fp8e4m3 with DoubleRow perf mode = 2 K-planes per PE pass).  The
uniform cubic B-spline bases are computed on-device in closed form via
truncated powers and the reflection B_j(u) = B_{7-j}(5-u): for
u = 2.5x + 2.5 in [0,5), T_q = relu(u-q)^3, T'_q = relu(5-q-u)^3,

  6*B7 = T4                     6*B0 = T'4
  6*B6 = T3 - 4*T4              6*B1 = T'3 - 4*T'4
  6*B5 = T2 - 4*T3 + 6*T4       6*B2 = T'2 - 4*T'3 + 6*T'4
  6*B4 = T1 - 4*T2 + 6*T3       6*B3 = T'1 - 4*T'2 + 6*T'3
         - 4*T4                        - 4*T'4

so every plane is a short chain of scalar_tensor_tensor MACs over
shared relu-cubes (no polynomial part, no interval masks), with a
uniform final scale 1/6 folded into paired fp8 casts on the scalar
engine.  Spline weights are pre-scaled x64 (fp8e4m3 subnormal
avoidance), base weights x64 in bf16, and the GELU applies scale=1/64.

Sharding: data-parallel over tokens (8192 rows -> 1024/core), weights
replicated.  Per core the 1024 tokens split into two 512-token chunks
(one PSUM bank each); emission is software-pipelined so feature builds
(ACT+DVE) run two chunks ahead of the PE-bound weight-stationary
sweeps, and the measurement path wraps the body in a tc.For_i hardware
loop (4 bodies per iteration) to amortize the ~27ms axon dispatch
overhead out of the timing.
"""

import sys

for _p in ("/opt/trn_rl_repo",):
    if _p not in sys.path:
        sys.path.insert(0, _p)

import numpy as np
import ml_dtypes

import concourse.bass as bass
import concourse.tile as tile
from concourse import bacc, mybir
from concourse.bass_utils import run_bass_kernel_spmd

AF = mybir.ActivationFunctionType
ALU = mybir.AluOpType
PM = mybir.MatmulPerfMode
DT = mybir.dt

N_CORES = 8
NTOK = 1024          # tokens per core
H = 1024             # input dim
D = 4096             # output dim
NB = 8               # number of basis functions
CHUNK = 512          # tokens per chunk (one PSUM bank)
NCHUNK = NTOK // CHUNK
DTI = D // 128       # 32 d-tiles
HT = H // 128        # 8 h-tiles
NG = HT * NB // 2    # 32 fp8 DoubleRow k-groups per d-tile
WSCALE = 64.0        # weight pre-scale (fp8 subnormal avoidance)

_NC_CACHE = {}


def _build_program(repeat=1):
    nc = bacc.Bacc("TRN2", target_bir_lowering=False, debug=False,
                   enable_asserts=False, num_devices=N_CORES)
    xt = nc.dram_tensor("xt", (H, NTOK), DT.float32, kind="ExternalInput").ap()
    wb = nc.dram_tensor("wb", (DTI, 128, HT * 128), DT.bfloat16,
                        kind="ExternalInput").ap()
    ws = nc.dram_tensor("ws", (DTI, 128, NG, 2, 128), DT.float8e4,
                        kind="ExternalInput").ap()
    out = nc.dram_tensor("out", (D, NTOK), DT.float32, kind="ExternalOutput").ap()

    f32 = DT.float32
    bf16 = DT.bfloat16
    f8 = DT.float8e4

    with tile.TileContext(nc) as tc:
        with (
            tc.tile_pool(name="constp", bufs=1) as constp,
            tc.tile_pool(name="xp", bufs=2) as xp,
            tc.tile_pool(name="sbp", bufs=2) as sbp,      # silu features
            tc.tile_pool(name="spp", bufs=2) as spp,      # spline features
            tc.tile_pool(name="scr", bufs=2) as scr,
            tc.tile_pool(name="wp", bufs=2) as wp,
            tc.tile_pool(name="psump", bufs=8, space=bass.MemorySpace.PSUM) as psump,
            tc.tile_pool(name="outp", bufs=4) as outp,
        ):
            # const APs for the Relu biases (2.5 - q)
            for v in (1.5, 0.5, -0.5, -1.5):
                cst = constp.tile([128, 1], f32, tag=f"c{v}", name="cst")
                nc.gpsimd.memset(cst[:], v)
                nc.const_aps.aps[(DT.float32, float(v))] = cst

            def cube_into(dst, xtile, sign, q):
                # dst = relu(sign*2.5*x + (2.5-q))^3
                r = scr.tile([128, CHUNK], f32, tag="r", name="r")
                nc.scalar.activation(r[:], xtile[:], AF.Relu,
                                     bias=2.5 - q, scale=sign * 2.5)
                r2 = scr.tile([128, CHUNK], f32, tag="r2", name="r2")
                nc.scalar.activation(r2[:], r[:], AF.Square)
                nc.vector.tensor_mul(dst, r2[:], r[:])

            def stt(dst, a, s, b):
                nc.vector.scalar_tensor_tensor(dst, a, s, b, ALU.mult, ALU.add)

            def build_feat(c):
                # ---- features: silu (bf16) + 8 spline planes (fp8) ----
                # Reflection: B_j(u) = B_{7-j}(5-u); with
                # T_q = relu(2.5x+2.5-q)^3, T'_q = relu(-2.5x+2.5-q)^3:
                #   B7=T4/6  B6=(T3-4T4)/6  B5=(T2-4T3+6T4)/6
                #   B4=(T1-4T2+6T3-4T4)/6  and B_{3-k} same in T'.
                silu_f, spl_f = [], []
                for ht in range(HT):
                    xtile = xp.tile([128, CHUNK], f32, tag="x", name="xtile")
                    nc.sync.dma_start(
                        xtile[:], xt[ht * 128:(ht + 1) * 128,
                                     c * CHUNK:(c + 1) * CHUNK])
                    sb_t = sbp.tile([128, CHUNK], bf16,
                                    tag=f"sb{ht}", name="sb_t")
                    nc.scalar.activation(sb_t[:], xtile[:], AF.Silu)
                    silu_f.append(sb_t)
                    sp_t = spp.tile([128, NB, CHUNK], f8,
                                    tag=f"sp{ht}", name="sp_t")
                    spl_f.append(sp_t)
                    acc = [scr.tile([128, 2, CHUNK], f32, tag=f"acc{i}",
                                    name="acc") for i in range(4)]
                    for sign, tip_pair, tip_slot, b1_dst, c2_acc, c3_acc \
                            in ((-1.0, 0, 0, (0, 1), 1, 1),
                                (1.0, 3, 1, (3, 0), 2, 2)):
                        # tip = T'4 (primed) or T4: lives in an acc slot
                        tip = acc[tip_pair][:, tip_slot, :]
                        cube_into(tip, xtile, sign, 4)
                        t3 = scr.tile([128, CHUNK], f32,
                                      tag=f"t3s{tip_slot}", name="t3")
                        cube_into(t3[:], xtile, sign, 3)
                        t2 = scr.tile([128, CHUNK], f32,
                                      tag=f"t2s{tip_slot}", name="t2")
                        cube_into(t2[:], xtile, sign, 2)
                        t1 = scr.tile([128, CHUNK], f32,
                                      tag=f"t1s{tip_slot}", name="t1")
                        cube_into(t1[:], xtile, sign, 1)
                        # B6-analog: -4*T4 + T3
                        b1 = acc[b1_dst[0]][:, b1_dst[1], :]
                        stt(b1, tip, -4.0, t3[:])
                        # B5-analog: 6T4 - 4T3 + T2 (2 steps)
                        b2 = acc[c2_acc][:, tip_slot, :]
                        stt(b2, tip, -1.5, t3[:])
                        stt(b2, b2, -4.0, t2[:])
                        # B4-analog: -4T4 + 6T3 - 4T2 + T1 (3 steps)
                        b3 = acc[c3_acc][:, 1 - tip_slot, :]
                        stt(b3, tip, -2.0 / 3.0, t3[:])
                        stt(b3, b3, -1.5, t2[:])
                        stt(b3, b3, -4.0, t1[:])
                    for pair in range(4):
                        nc.scalar.activation(sp_t[:, 2 * pair:2 * pair + 2, :],
                                             acc[pair][:], AF.Copy,
                                             bias=0.0, scale=1.0 / 6.0)
                return silu_f, spl_f

            def sweep(c, feats):
                # ---- weight-stationary matmul sweep for one chunk ----
                silu_f, spl_f = feats
                for di in range(DTI):
                    wb_t = wp.tile([128, HT * 128], bf16, tag="wb",
                                   name="wb_t")
                    nc.sync.dma_start(wb_t[:], wb[di])
                    ws_t = wp.tile([128, NG, 2, 128], f8, tag="ws",
                                   name="ws_t")
                    half = NG // 2
                    nc.sync.dma_start(ws_t[:, :half, :, :], ws[di, :, :half])
                    nc.sync.dma_start(ws_t[:, half:, :, :], ws[di, :, half:])
                    ps = psump.tile([128, CHUNK], f32, tag="ps", name="ps")
                    for ht in range(HT):
                        nc.tensor.matmul(
                            ps[:], wb_t[:, ht * 128:(ht + 1) * 128],
                            silu_f[ht][:],
                            start=(ht == 0), stop=False)
                    for g in range(NG):
                        ht, t = g // 4, g % 4
                        nc.tensor.matmul(
                            ps[:], ws_t[:, g, :, :],
                            spl_f[ht][:, 2 * t:2 * t + 2, :],
                            start=False, stop=(g == NG - 1),
                            perf_mode=PM.DoubleRow)
                    ot = outp.tile([128, CHUNK], f32, tag="o", name="ot")
                    nc.scalar.activation(ot[:], ps[:], AF.Gelu,
                                         scale=1.0 / WSCALE)
                    nc.sync.dma_start(
                        out[di * 128:(di + 1) * 128,
                            c * CHUNK:(c + 1) * CHUNK], ot[:])

            def stream(n_bodies):
                # software-pipelined emission: features run 2 chunks ahead of
                # the sweeps so ACT/DVE feature work for chunk k+1/k+2 sits
                # ahead of sweep k's GELUs in the engine queues and overlaps
                # the PE-bound sweep.
                cols = [c for _ in range(n_bodies) for c in range(NCHUNK)]
                feats = {k: build_feat(cols[k]) for k in range(min(2, len(cols)))}
                for k in range(len(cols)):
                    sweep(cols[k], feats.pop(k))
                    if k + 2 < len(cols):
                        feats[k + 2] = build_feat(cols[k + 2])

            if repeat == 1:
                stream(1)
            elif repeat % 8 == 0:
                # 8 bodies per hw-loop iteration: the loop boundary barrier
                # costs ~86us of lost overlap; amortize it over 8 bodies
                with tc.For_i(0, repeat // 8):
                    stream(8)
            elif repeat % 4 == 0:
                with tc.For_i(0, repeat // 4):
                    stream(4)
            else:
                with tc.For_i(0, repeat):
                    stream(1)

    nc.compile()
    return nc


def _prep_weights(base_weight, spline_weight, spline_scaler):
    # base: [di, p(h within tile), ht*128 + m(d within tile)], x64, bf16
    wb = (base_weight * WSCALE).reshape(DTI, 128, HT, 128) \
        .transpose(0, 3, 2, 1).reshape(DTI, 128, HT * 128)
    wb = np.ascontiguousarray(wb.astype(ml_dtypes.bfloat16))
    # spline: [di, p, g=(ht*4+t), i, m] with basis j = 2t+i, x64, fp8e4m3
    sw = (spline_weight * spline_scaler[..., None] * WSCALE) \
        .reshape(DTI, 128, HT, 128, 4, 2).transpose(0, 3, 2, 4, 5, 1)
    sw = np.ascontiguousarray(
        sw.reshape(DTI, 128, NG, 2, 128).astype(ml_dtypes.float8_e4m3))
    return wb, sw


def _prep_in_maps(x, base_weight, spline_weight, spline_scaler):
    xf = np.asarray(x, np.float32).reshape(N_CORES * NTOK, H)
    wb, ws = _prep_weights(np.asarray(base_weight, np.float32),
                           np.asarray(spline_weight, np.float32),
                           np.asarray(spline_scaler, np.float32))
    in_maps = []
    for c in range(N_CORES):
        xs = np.ascontiguousarray(xf[c * NTOK:(c + 1) * NTOK].T)  # (H, NTOK)
        in_maps.append({"xt": xs, "wb": wb, "ws": ws})
    return in_maps


def kernel(x, base_weight, spline_weight, spline_scaler, _trace=False):
    if "nc" not in _NC_CACHE:
        _NC_CACHE["nc"] = _build_program()
    nc = _NC_CACHE["nc"]

    in_maps = _prep_in_maps(x, base_weight, spline_weight, spline_scaler)
    res = run_bass_kernel_spmd(nc, in_maps, core_ids=list(range(N_CORES)),
                               trace=_trace)
    full = np.concatenate([res.results[c]["out"] for c in range(N_CORES)],
                          axis=1)               # (4096, 8192)
    out = np.ascontiguousarray(full.T).reshape(x.shape[0], x.shape[1], D)
    if _trace:
        kernel.last_exec_time_ns = res.exec_time_ns
        kernel.last_results = res
    return out.astype(np.float32, copy=False)


def measure_exec_ns(inputs, n=5, repeat=8):
    """Amortized on-device execution time: the kernel body is unrolled
    `repeat` times inside one NEFF; each wall-clock sample of a full
    dispatch is divided by `repeat`, and the min over samples is
    reported.  wall = dispatch_overhead + repeat * t_kernel, so
    wall/repeat = t_kernel + overhead/repeat — still an upper bound on
    the true per-execution HW time, but far tighter than a single-shot
    call through the (high-latency, high-variance) axon tunnel."""
    import time
    import jax
    from jax.sharding import Mesh, PartitionSpec, NamedSharding
    try:
        from jax.experimental.shard_map import shard_map
    except ImportError:
        from jax.shard_map import shard_map
    from concourse.bass2jax import (_bass_exec_p, install_neuronx_cc_hook,
                                    partition_id_tensor)

    key = f"nc{repeat}"
    if key not in _NC_CACHE:
        _NC_CACHE[key] = _build_program(repeat=repeat)
    nc = _NC_CACHE[key]
    install_neuronx_cc_hook()

    pname = (nc.partition_id_tensor.name if nc.partition_id_tensor else None)
    in_names, out_names, out_avals, zero_outs = [], [], [], []
    for alloc in nc.m.functions[0].allocations:
        if not isinstance(alloc, mybir.MemoryLocationSet):
            continue
        name = alloc.memorylocations[0].name
        if alloc.kind == "ExternalInput":
            if name != pname:
                in_names.append(name)
        elif alloc.kind == "ExternalOutput":
            out_names.append(name)
            shape = tuple(alloc.tensor_shape)
            dtype = mybir.dt.np(alloc.dtype)
            out_avals.append(jax.core.ShapedArray(shape, dtype))
            zero_outs.append(np.zeros(shape, dtype))
    n_params = len(in_names)
    all_in = in_names + out_names
    if pname is not None:
        all_in = all_in + [pname]

    def _body(*args):
        operands = list(args)
        if pname is not None:
            operands.append(partition_id_tensor())
        outs = _bass_exec_p.bind(
            *operands, out_avals=tuple(out_avals), in_names=tuple(all_in),
            out_names=tuple(out_names), lowering_input_output_aliases=(),
            sim_require_finite=True, sim_require_nnan=True, nc=nc)
        return tuple(outs)

    in_maps = _prep_in_maps(inputs["x"], inputs["base_weight"],
                            inputs["spline_weight"], inputs["spline_scaler"])
    per_core = {nm: [in_maps[c][nm] for c in range(N_CORES)]
                for nm in in_names}
    devices = jax.devices()[:N_CORES]
    mesh = Mesh(np.asarray(devices), ("core",))
    sh = NamedSharding(mesh, PartitionSpec("core"))
    in_specs = (PartitionSpec("core"),) * (n_params + len(out_names))
    out_specs = (PartitionSpec("core"),) * len(out_names)
    fn = jax.jit(shard_map(_body, mesh=mesh, in_specs=in_specs,
                           out_specs=out_specs, check_rep=False),
                 keep_unused=True)
    concat_in = [jax.device_put(
        np.concatenate(per_core[name], axis=0), sh) for name in in_names]
    zeros = [jax.device_put(
        np.zeros((N_CORES * z.shape[0], *z.shape[1:]), z.dtype), sh)
        for z in zero_outs]
    for a in concat_in + zeros:
        a.block_until_ready()
    times = []
    for trial in range(n):
        if trial > 0:
            # cooldown: back-to-back calls throttle ~15-20% after the first;
            # a gap lets each sample start from the cool-device state (the
            # first timed call gets extra to shed the warmup call's heat)
            time.sleep(15.0 if trial == 1 else 8.0)
        t0 = time.perf_counter()
        outs = fn(*concat_in, *zeros)
        for o in outs:
            o.block_until_ready()
        dt_s = time.perf_counter() - t0
        if trial > 0:        # first call includes compile
            times.append(dt_s)
    print(f"  [repeat={repeat}] per-call ms:",
          [f"{t*1e3:.2f}" for t in times])
    return int(min(times) * 1e9 / repeat)
